# revision 38
# baseline (speedup 1.0000x reference)
# Trainium2 Bass kernel for nn_EqPropNetwork (equilibrium-propagation relaxation).
#
# Math (per reference.py):
#   c_h = x @ W1 + b1                                  [B, HID]  (constant)
#   repeat T times:
#     h' = clip(0.5*h + 0.5*c_h + 0.5*(y @ W2.T), 0, 1)
#     y' = clip(0.25*y + 0.5*(h @ W2) + 0.5*b2 + 0.25*onehot(target), 0, 1)
#   out = concat(h, y)
#
# Per-core mapping (B_loc = 4096, pure data parallel over 8 cores):
#   * h-state feature-major: s[c] = (h + c_h) chunk  [128 feat, 4096 batch] fp16.
#     Per step, per [128,1024] psum tile: u = 0.5*s (identity matmul)
#     + 0.5*y@W2T (psi matmuls consuming the transposed-y tile), then
#     s' = clip(u) + c_h via relu/min/add spread across ACT/DVE/Pool.
#   * y-state batch-major: psum_y [128, 32*10] accumulates, per batch tile bt,
#     0.5*s@W2 (s-slice stationary, out free = 10) + 0.25*y + dbar, where
#     dbar = 0.25*onehot + 0.5*b2 - 0.5*(c_h@W2).  One DVE clip writes the
#     padded batch-major y; 8 DMA xbar transposes produce the feature-major
#     yT4 tiles the next step's psi/y-carry matmuls consume.
import sys

import numpy as np

if "/opt/trn_rl_repo" not in sys.path:
    sys.path.insert(0, "/opt/trn_rl_repo")

N_CORES = 8
B, IN, HID, OUT = 32768, 784, 512, 10
BLOC = B // N_CORES          # 4096
HCH = HID // 128             # 4 feature chunks
KIN = 7                      # IN chunks of 112
KC = IN // KIN               # 112
NBT = BLOC // 128            # 32 batch tiles
NPC = NBT // 4               # 8 transpose pieces / psi col groups

# cst16 column offsets
C_HALFI = 0        # [128,128] 0.5*I
C_W2T2R = 128      # [128,512] rows 32t+o = 0.5*W2[f,o], cols f
C_W2C2 = 640       # 4 x [128,10] 0.5*W2 chunks
C_I10Q = 680       # [128,10] rows 32t+o: 0.25 at col o
C_I10 = 690        # [128,10] rows o: 1.0 at col o
C_I128 = 704       # [128,128] identity (for PE transposes)
CF16_W = 832

_BUILT = {}


def _build(T):
    from concourse import bacc, mybir
    from concourse.tile import TileContext

    f32 = mybir.dt.float32
    f16 = mybir.dt.float16
    Alu = mybir.AluOpType
    Act = mybir.ActivationFunctionType

    nc = bacc.Bacc("TRN2", target_bir_lowering=False)

    xT16 = nc.declare_dram_parameter("xT16", [IN, BLOC], f16, isOutput=False)
    hT16 = nc.declare_dram_parameter("hT16", [HID, BLOC], f16, isOutput=False)
    y0T4 = nc.declare_dram_parameter("y0T4", [128, NPC * 128], f16, isOutput=False)
    dbar0B = nc.declare_dram_parameter("dbar0B", [128, NBT * OUT], f16,
                                       isOutput=False)
    w116 = nc.declare_dram_parameter("w116", [KC, KIN * HID], f16, isOutput=False)
    cst16 = nc.declare_dram_parameter("cst16", [128, CF16_W], f16, isOutput=False)
    cst32 = nc.declare_dram_parameter("cst32", [128, 8], f32, isOutput=False)

    hTo16 = nc.declare_dram_parameter("hTo16", [HID, BLOC], f16, isOutput=True)
    ybo16 = nc.declare_dram_parameter("ybo16", [128, NBT * OUT], f16, isOutput=True)

    with TileContext(nc) as tc:
        with (
            tc.tile_pool(name="const", bufs=1) as constp,
            tc.tile_pool(name="sp", bufs=2) as sp,
            tc.tile_pool(name="ypool", bufs=2) as yp,
            tc.tile_pool(name="tmp", bufs=3) as tmpp,
        ):
            cf16 = constp.tile([128, CF16_W], f16, tag="cf16", name="cf16")
            cf32 = constp.tile([128, 8], f32, tag="cf32", name="cf32")
            ch = constp.tile([128, HCH * BLOC], f16, tag="ch", name="ch")
            dbarB = constp.tile([128, NBT * OUT], f16, tag="dbarB",
                                name="dbarB")
            db0 = constp.tile([128, NBT * OUT], f16, tag="db0", name="db0")
            nc.sync.dma_start(out=db0[:], in_=dbar0B[:])
            # cf16/cf32 are loaded after the first GEMM-critical chunks
            # (see below) so the first matmul isn't queued behind them.

            halfI = cf16[:, C_HALFI:C_HALFI + 128]
            w2t2r = cf16[:, C_W2T2R:C_W2T2R + HID]
            w2c2 = [cf16[:, C_W2C2 + OUT * c:C_W2C2 + OUT * (c + 1)]
                    for c in range(HCH)]
            i10q = cf16[:, C_I10Q:C_I10Q + OUT]
            i10 = cf16[0:OUT, C_I10:C_I10 + OUT]
            i128 = cf16[:, C_I128:C_I128 + 128]
            idxcol = cf32[:, 0:1]
            b2col = cf32[:, 1:2]
            b1c = [cf32[:, 2 + c:3 + c] for c in range(HCH)]

            chv = [ch[:, BLOC * c:BLOC * (c + 1)] for c in range(HCH)]

            # ---------- setup: c_h = x@W1 + b1 (feature-major), s0, dbar ----
            s_cur = [sp.tile([128, BLOC], f16, tag=f"s{c}", name=f"s{c}")
                     for c in range(HCH)]
            with (
                tc.tile_pool(name="xstage", bufs=1) as xst,
                tc.tile_pool(name="w1p", bufs=1) as w1p,
                tc.tile_pool(name="dstage", bufs=2) as dst,
                tc.tile_pool(name="pS", bufs=8, space="PSUM") as pS,
            ):
                w1t = w1p.tile([128, KIN * HID], f16, tag="w1", name="w1")
                xg = xst.tile([128, KIN * BLOC], f16, tag="xg", name="xg")
                # k-outer GEMM: 8 open psum tiles per 1024-col group, so
                # matmuls start as soon as the first w1/x chunks land.
                for k in range(KIN):
                    nc.sync.dma_start(
                        out=w1t[0:KC, HID * k:HID * (k + 1)],
                        in_=w116[:, HID * k:HID * (k + 1)],
                    )
                    nc.sync.dma_start(
                        out=xg[0:KC, BLOC * k:BLOC * k + 1024],
                        in_=xT16[KC * k:KC * (k + 1), 0:1024],
                    )
                nc.sync.dma_start(out=cf16[:], in_=cst16[:])
                nc.sync.dma_start(out=cf32[:], in_=cst32[:])
                for gg in range(1, 4):
                    for k in range(KIN):
                        nc.sync.dma_start(
                            out=xg[0:KC, BLOC * k + 1024 * gg:
                                   BLOC * k + 1024 * (gg + 1)],
                            in_=xT16[KC * k:KC * (k + 1),
                                     1024 * gg:1024 * (gg + 1)],
                        )
                hstg = [None] * HCH
                for c in range(HCH):
                    hstg[c] = sp.tile([128, BLOC], f16, tag=f"s{c}",
                                      name=f"h{c}")
                    nc.sync.dma_start(
                        out=hstg[c][:], in_=hT16[128 * c:128 * (c + 1), :]
                    )
                for gg in range(4):
                    qc = slice(1024 * gg, 1024 * (gg + 1))
                    pss = [pS.tile([128, 512], f32, tag="pS", name="pS")
                           for _ in range(8)]
                    for k in range(KIN):
                        for c in range(HCH):
                            for g2 in range(2):
                                nc.tensor.matmul(
                                    pss[2 * c + g2][:],
                                    w1t[0:KC, HID * k + 128 * c:
                                        HID * k + 128 * (c + 1)],
                                    xg[0:KC, BLOC * k + 1024 * gg + 512 * g2:
                                       BLOC * k + 1024 * gg + 512 * (g2 + 1)],
                                    start=(k == 0),
                                    stop=(k == KIN - 1),
                                    tile_position=(0, 0),
                                )
                    for c in range(HCH):
                        for g2 in range(2):
                            nc.scalar.activation(
                                chv[c][:, 1024 * gg + 512 * g2:
                                       1024 * gg + 512 * (g2 + 1)],
                                pss[2 * c + g2][:],
                                Act.Identity,
                                bias=b1c[c],
                                scale=1.0,
                            )
                    # s0 = h0 + c_h for this column group
                    for c in range(HCH):
                        nc.vector.tensor_tensor(
                            s_cur[c][:, qc], hstg[c][:, qc], chv[c][:, qc],
                            Alu.add,
                        )

                # dbarB (batch-major): 0.5*(c_h@W2) per batch tile,
                # then dbarB = db0 - that.  After the GEMM so the psum pool
                # rotation never stalls a column group.
                for gg in range(4):
                    ps2 = pS.tile([128, 512], f32, tag="pS", name="pS")
                    for bt in range(8 * gg, 8 * gg + 8):
                        slb = ps2[:, OUT * (bt - 8 * gg):OUT * (bt - 8 * gg + 1)]
                        for c in range(HCH):
                            nc.tensor.matmul(
                                slb,
                                chv[c][:, 128 * bt:128 * (bt + 1)],
                                w2c2[c],
                                start=(c == 0),
                                stop=(c == HCH - 1),
                                tile_position=(0, 0),
                            )
                    nc.vector.tensor_tensor(
                        dbarB[:, OUT * 8 * gg:OUT * 8 * (gg + 1)],
                        db0[:, OUT * 8 * gg:OUT * 8 * (gg + 1)],
                        ps2[:, 0:OUT * 8],
                        Alu.subtract,
                    )

            # y transpose tiles: [128, NPC*128], piece pc rows 32t+o,
            # cols p  <->  y[128*(4*pc+t)+p, o]
            yT4 = yp.tile([128, NPC * 128], f16, tag="yT4", name="yT4")
            nc.sync.dma_start(out=yT4[:], in_=y0T4[:])
            ybpad0 = yp.tile([128, NBT * 32], f16, tag="ybpad", name="ybpad")
            nc.vector.memset(ybpad0[:], 0.0)
            ybpad1 = yp.tile([128, NBT * 32], f16, tag="ybpad", name="ybpad")
            nc.vector.memset(ybpad1[:], 0.0)

            # EW path per (c, gg) tile index i = 4*c+gg:
            #   0..7  -> A1: ACT relu, DVE min, DVE add
            #   8..11 -> A2: ACT relu, Pool (min, add)
            #   12..15-> D:  DVE clip-from-psum, Pool (add)
            # ---------- relaxation loop ----------
            with (
                tc.tile_pool(name="pU", bufs=3, space="PSUM") as pU,
                tc.tile_pool(name="pY", bufs=1, space="PSUM") as pY,
                tc.tile_pool(name="pT", bufs=1, space="PSUM") as pT,
            ):
              for t in range(T):
                last = t == T - 1
                h_chunks = (0, 1, 2)
                # ---- h update (chunks 0-2 first; y-phase sits between
                # so the in-order PE queue never head-of-line blocks) ----
                s_next = (
                    None if last else
                    [sp.tile([128, BLOC], f16, tag=f"s{c}", name=f"s{c}")
                     for c in range(HCH)]
                )
                hout = (
                    [sp.tile([128, BLOC], f16, tag=f"s{c}", name=f"s{c}")
                     for c in range(HCH)] if last else None
                )
                for c in h_chunks:
                    for gg in range(4):
                        cols = slice(1024 * gg, 1024 * (gg + 1))
                        pu = pU.tile([128, 1024], f32, tag="pU", name="pU")
                        for t4 in range(8):
                            bt = 8 * gg + t4
                            band = 32 * (bt % 4)
                            ypc = bt // 4
                            nc.tensor.matmul(
                                pu[:, 128 * t4:128 * (t4 + 1)], halfI,
                                s_cur[c][:, 1024 * gg + 128 * t4:
                                          1024 * gg + 128 * (t4 + 1)],
                                start=True, stop=False, tile_position=(0, 0),
                            )
                            nc.tensor.matmul(
                                pu[:, 128 * t4:128 * (t4 + 1)],
                                w2t2r[band:band + OUT,
                                      128 * c:128 * (c + 1)],
                                yT4[band:band + OUT,
                                    128 * ypc:128 * (ypc + 1)],
                                start=False, stop=True,
                                tile_position=(band, 0),
                            )
                        i = 4 * c + gg
                        if last:
                            r = tmpp.tile([128, 1024], f16, tag="tmp",
                                          name="tmp")
                            if i % 2 == 0:
                                nc.scalar.activation(r[:], pu[:], Act.Relu)
                                nc.vector.tensor_scalar_min(
                                    hout[c][:, cols], r[:], 1.0
                                )
                            else:
                                nc.vector.tensor_scalar(
                                    hout[c][:, cols], pu[:], 0.0, 1.0,
                                    Alu.max, Alu.min,
                                )
                        elif i % 4 != 3:
                            # clip(u) into s_next; c_h added by accum-DMA below
                            r = tmpp.tile([128, 1024], f16, tag="tmp",
                                          name="tmp")
                            nc.scalar.activation(r[:], pu[:], Act.Relu)
                            nc.vector.tensor_scalar_min(
                                s_next[c][:, cols], r[:], 1.0
                            )
                        else:
                            nc.vector.tensor_scalar(
                                s_next[c][:, cols], pu[:], 0.0, 1.0,
                                Alu.max, Alu.min,
                            )
                        if not last and gg % 2 == 1:
                            # s_next += c_h over the finished 2048-col pair,
                            # on otherwise-idle DMA engines (SWDGE accum)
                            pair = slice(1024 * (gg - 1), 1024 * (gg + 1))
                            nc.gpsimd.dma_start(
                                out=s_next[c][:, pair],
                                in_=chv[c][:, pair],
                                accum_op=Alu.add,
                            )
                # ---- y update (reads s_cur, yT4) ----
                pyy = pY.tile([128, NBT * OUT], f32, tag="pY", name="pY")
                for bt in range(NBT):
                    sl = pyy[:, OUT * bt:OUT * (bt + 1)]
                    band = 32 * (bt % 4)
                    ypc = bt // 4
                    for c in range(HCH):
                        nc.tensor.matmul(
                            sl,
                            s_cur[c][:, 128 * bt:128 * (bt + 1)],
                            w2c2[c],
                            start=(c == 0),
                            stop=False,
                            tile_position=(0, 0),
                        )
                    nc.tensor.matmul(
                        sl,
                        yT4[band:band + OUT, 128 * ypc:128 * (ypc + 1)],
                        i10q[band:band + OUT, :],
                        start=False,
                        stop=False,
                        tile_position=(band, 0),
                    )
                    nc.tensor.matmul(
                        sl,
                        i128,
                        dbarB[:, OUT * bt:OUT * (bt + 1)],
                        start=False,
                        stop=True,
                        tile_position=(0, 0),
                    )
                if last:
                    ybo = tmpp.tile([128, NBT * OUT], f16, tag="ybo",
                                    name="ybo", bufs=1)
                    nc.vector.tensor_scalar(
                        ybo[:], pyy[:], 0.0, 1.0, Alu.max, Alu.min
                    )
                    nc.sync.dma_start(out=ybo16[:], in_=ybo[:])
                    yT4n = None
                else:
                    ybp = yp.tile([128, NBT * 32], f16, tag="ybpad",
                                  name="ybpad")
                    nc.vector.tensor_scalar(
                        ybp.rearrange("p (b o) -> p b o", o=32)[:, :, 0:OUT],
                        pyy.rearrange("p (b o) -> p b o", o=OUT),
                        0.0, 1.0, Alu.max, Alu.min,
                    )
                    yT4n = yp.tile([128, NPC * 128], f16, tag="yT4",
                                   name="yT4")
                    for wave in range(2):
                        pt = pT.tile([128, 512], f16, tag="pT", name="pT")
                        for j in range(4):
                            pc = 4 * wave + j
                            nc.tensor.transpose(
                                pt[:, 128 * j:128 * (j + 1)],
                                ybp[:, 128 * pc:128 * (pc + 1)],
                                i128,
                            )
                        if wave == 0:
                            nc.scalar.copy(yT4n[:, 0:512], pt[:])
                        else:
                            nc.vector.tensor_copy(yT4n[:, 512:1024], pt[:])

                h_chunks = (3,)
                for c in h_chunks:
                    for gg in range(4):
                        cols = slice(1024 * gg, 1024 * (gg + 1))
                        pu = pU.tile([128, 1024], f32, tag="pU", name="pU")
                        for t4 in range(8):
                            bt = 8 * gg + t4
                            band = 32 * (bt % 4)
                            ypc = bt // 4
                            nc.tensor.matmul(
                                pu[:, 128 * t4:128 * (t4 + 1)], halfI,
                                s_cur[c][:, 1024 * gg + 128 * t4:
                                          1024 * gg + 128 * (t4 + 1)],
                                start=True, stop=False, tile_position=(0, 0),
                            )
                            nc.tensor.matmul(
                                pu[:, 128 * t4:128 * (t4 + 1)],
                                w2t2r[band:band + OUT,
                                      128 * c:128 * (c + 1)],
                                yT4[band:band + OUT,
                                    128 * ypc:128 * (ypc + 1)],
                                start=False, stop=True,
                                tile_position=(band, 0),
                            )
                        i = 4 * c + gg
                        if last:
                            r = tmpp.tile([128, 1024], f16, tag="tmp",
                                          name="tmp")
                            if i % 2 == 0:
                                nc.scalar.activation(r[:], pu[:], Act.Relu)
                                nc.vector.tensor_scalar_min(
                                    hout[c][:, cols], r[:], 1.0
                                )
                            else:
                                nc.vector.tensor_scalar(
                                    hout[c][:, cols], pu[:], 0.0, 1.0,
                                    Alu.max, Alu.min,
                                )
                        elif i % 4 != 3:
                            # clip(u) into s_next; c_h added by accum-DMA below
                            r = tmpp.tile([128, 1024], f16, tag="tmp",
                                          name="tmp")
                            nc.scalar.activation(r[:], pu[:], Act.Relu)
                            nc.vector.tensor_scalar_min(
                                s_next[c][:, cols], r[:], 1.0
                            )
                        else:
                            nc.vector.tensor_scalar(
                                s_next[c][:, cols], pu[:], 0.0, 1.0,
                                Alu.max, Alu.min,
                            )
                        if not last and gg % 2 == 1:
                            # s_next += c_h over the finished 2048-col pair,
                            # on otherwise-idle DMA engines (SWDGE accum)
                            pair = slice(1024 * (gg - 1), 1024 * (gg + 1))
                            nc.gpsimd.dma_start(
                                out=s_next[c][:, pair],
                                in_=chv[c][:, pair],
                                accum_op=Alu.add,
                            )
                if last:
                    for c in range(HCH):
                        for hf in range(2):
                            nc.sync.dma_start(
                                out=hTo16[128 * c:128 * (c + 1),
                                          2048 * hf:2048 * (hf + 1)],
                                in_=hout[c][:, 2048 * hf:2048 * (hf + 1)],
                            )
                else:
                    s_cur = s_next
                    yT4 = yT4n

    if not nc.is_finalized():
        nc.finalize()
    return nc


def _consts(W1, W2, b1, b2):
    cst16 = np.zeros((128, CF16_W), dtype=np.float16)
    cst16[:, C_HALFI:C_HALFI + 128] = 0.5 * np.eye(128, dtype=np.float16)
    w2t2 = (0.5 * W2.T).astype(np.float16)          # [10, 512]
    w2c2 = (0.5 * W2).astype(np.float16)            # [512, 10]
    for t in range(4):
        cst16[32 * t:32 * t + OUT, C_W2T2R:C_W2T2R + HID] = w2t2
        for o in range(OUT):
            cst16[32 * t + o, C_I10Q + o] = 0.25
    for c in range(HCH):
        cst16[:, C_W2C2 + OUT * c:C_W2C2 + OUT * (c + 1)] = \
            w2c2[128 * c:128 * (c + 1), :]
    for o in range(OUT):
        cst16[o, C_I10 + o] = 1.0
    cst16[:, C_I128:C_I128 + 128] = np.eye(128, dtype=np.float16)
    cst32 = np.zeros((128, 8), dtype=np.float32)
    cst32[:, 0] = -1.0
    cst32[0:OUT, 0] = np.arange(OUT, dtype=np.float32)
    cst32[0:OUT, 1] = 0.5 * b2
    for c in range(HCH):
        cst32[:, 2 + c] = b1[128 * c:128 * (c + 1)]
    return cst16, cst32


def kernel(**inputs):
    from concourse import bass_utils

    x = np.asarray(inputs["x"], dtype=np.float32)
    h0 = np.asarray(inputs["h_init"], dtype=np.float32)
    y0 = np.asarray(inputs["y_init"], dtype=np.float32)
    W1 = np.asarray(inputs["W1"], dtype=np.float32)
    W2 = np.asarray(inputs["W2"], dtype=np.float32)
    b1 = np.asarray(inputs["b1"], dtype=np.float32).reshape(HID)
    b2 = np.asarray(inputs["b2"], dtype=np.float32).reshape(OUT)
    target = np.asarray(inputs["target"]).astype(np.int64)
    T = int(inputs["T"])

    xT16 = np.ascontiguousarray(x.T.astype(np.float16))       # [IN, B]
    hT16 = np.ascontiguousarray(h0.T.astype(np.float16))      # [HID, B]
    y016 = y0.astype(np.float16)                              # [B, OUT]
    # w116 layout: [112, 7*512], chunk k at cols 512k
    w116 = np.zeros((KC, KIN * HID), dtype=np.float16)
    for k in range(KIN):
        w116[:, HID * k:HID * (k + 1)] = W1[KC * k:KC * (k + 1), :]

    cst16, cst32 = _consts(W1, W2, b1, b2)

    key = T
    if key not in _BUILT:
        _BUILT[key] = _build(T)
    nc = _BUILT[key]

    in_maps = []
    for core in range(N_CORES):
        sl = slice(core * BLOC, (core + 1) * BLOC)
        yc = y016[sl]                                         # [BLOC, 10]
        # y0T4[32t+o, 128*pc+p] = y[128*(4*pc+t)+p, o]
        y0T4 = np.zeros((128, NPC * 128), dtype=np.float16)
        for pc in range(NPC):
            for tt in range(4):
                bt = 4 * pc + tt
                y0T4[32 * tt:32 * tt + OUT, 128 * pc:128 * (pc + 1)] = \
                    yc[128 * bt:128 * (bt + 1), :].T
        dbar0B = np.zeros((128, NBT * OUT), dtype=np.float16)
        tgt0 = target[sl]
        for bt in range(NBT):
            oh = np.zeros((128, OUT), np.float32)
            oh[np.arange(128), tgt0[128 * bt:128 * (bt + 1)]] = 1.0
            dbar0B[:, OUT * bt:OUT * (bt + 1)] = \
                (0.25 * oh + 0.5 * b2).astype(np.float16)
        in_maps.append({
            "xT16": np.ascontiguousarray(xT16[:, sl]),
            "hT16": np.ascontiguousarray(hT16[:, sl]),
            "y0T4": y0T4,
            "dbar0B": dbar0B,
            "w116": w116,
            "cst16": cst16,
            "cst32": cst32,
        })

    res = bass_utils.run_bass_kernel_spmd(nc, in_maps, list(range(N_CORES)))
    globals()["_LAST_RESULTS"] = res

    out = np.empty((B, HID + OUT), dtype=np.float32)
    for core in range(N_CORES):
        sl = slice(core * BLOC, (core + 1) * BLOC)
        out[sl, :HID] = np.asarray(res.results[core]["hTo16"]).T
        yb = np.asarray(res.results[core]["ybo16"])           # [128, 32*10]
        out[sl, HID:] = (
            yb.reshape(128, NBT, OUT).transpose(1, 0, 2).reshape(BLOC, OUT)
        )
    return out


# revision 39
# speedup vs baseline: 1.0006x; 1.0006x over previous
# Trainium2 Bass kernel for nn_EqPropNetwork (equilibrium-propagation relaxation).
#
# Math (per reference.py):
#   c_h = x @ W1 + b1                                  [B, HID]  (constant)
#   repeat T times:
#     h' = clip(0.5*h + 0.5*c_h + 0.5*(y @ W2.T), 0, 1)
#     y' = clip(0.25*y + 0.5*(h @ W2) + 0.5*b2 + 0.25*onehot(target), 0, 1)
#   out = concat(h, y)
#
# Per-core mapping (B_loc = 4096, pure data parallel over 8 cores):
#   * h-state feature-major: s[c] = (h + c_h) chunk  [128 feat, 4096 batch] fp16.
#     Per step, per [128,1024] psum tile: u = 0.5*s (identity matmul)
#     + 0.5*y@W2T (psi matmuls consuming the transposed-y tile), then
#     s' = clip(u) + c_h via relu/min/add spread across ACT/DVE/Pool.
#   * y-state batch-major: psum_y [128, 32*10] accumulates, per batch tile bt,
#     0.5*s@W2 (s-slice stationary, out free = 10) + 0.25*y + dbar, where
#     dbar = 0.25*onehot + 0.5*b2 - 0.5*(c_h@W2).  One DVE clip writes the
#     padded batch-major y; 8 DMA xbar transposes produce the feature-major
#     yT4 tiles the next step's psi/y-carry matmuls consume.
import sys

import numpy as np

if "/opt/trn_rl_repo" not in sys.path:
    sys.path.insert(0, "/opt/trn_rl_repo")

N_CORES = 8
B, IN, HID, OUT = 32768, 784, 512, 10
BLOC = B // N_CORES          # 4096
HCH = HID // 128             # 4 feature chunks
KIN = 7                      # IN chunks of 112
KC = IN // KIN               # 112
NBT = BLOC // 128            # 32 batch tiles
NPC = NBT // 4               # 8 transpose pieces / psi col groups

# cst16 column offsets
C_HALFI = 0        # [128,128] 0.5*I
C_W2T2R = 128      # [128,512] rows 32t+o = 0.5*W2[f,o], cols f
C_W2C2 = 640       # 4 x [128,10] 0.5*W2 chunks
C_I10Q = 680       # [128,10] rows 32t+o: 0.25 at col o
C_I10 = 690        # [128,10] rows o: 1.0 at col o
C_I128 = 704       # [128,128] identity (for PE transposes)
CF16_W = 832

_BUILT = {}


def _build(T):
    from concourse import bacc, mybir
    from concourse.tile import TileContext

    f32 = mybir.dt.float32
    f16 = mybir.dt.float16
    Alu = mybir.AluOpType
    Act = mybir.ActivationFunctionType

    nc = bacc.Bacc("TRN2", target_bir_lowering=False)

    xT16 = nc.declare_dram_parameter("xT16", [IN, BLOC], f16, isOutput=False)
    hT16 = nc.declare_dram_parameter("hT16", [HID, BLOC], f16, isOutput=False)
    y0T4 = nc.declare_dram_parameter("y0T4", [128, NPC * 128], f16, isOutput=False)
    dbar0B = nc.declare_dram_parameter("dbar0B", [128, NBT * OUT], f16,
                                       isOutput=False)
    w116 = nc.declare_dram_parameter("w116", [KC, KIN * HID], f16, isOutput=False)
    cst16 = nc.declare_dram_parameter("cst16", [128, CF16_W], f16, isOutput=False)
    cst32 = nc.declare_dram_parameter("cst32", [128, 8], f32, isOutput=False)

    hTo16 = nc.declare_dram_parameter("hTo16", [HID, BLOC], f16, isOutput=True)
    ybo16 = nc.declare_dram_parameter("ybo16", [128, NBT * OUT], f16, isOutput=True)

    with TileContext(nc) as tc:
        with (
            tc.tile_pool(name="const", bufs=1) as constp,
            tc.tile_pool(name="sp", bufs=2) as sp,
            tc.tile_pool(name="ypool", bufs=2) as yp,
            tc.tile_pool(name="tmp", bufs=3) as tmpp,
        ):
            cf16 = constp.tile([128, CF16_W], f16, tag="cf16", name="cf16")
            cf32 = constp.tile([128, 8], f32, tag="cf32", name="cf32")
            ch = constp.tile([128, HCH * BLOC], f16, tag="ch", name="ch")
            dbarB = constp.tile([128, NBT * OUT], f16, tag="dbarB",
                                name="dbarB")
            db0 = constp.tile([128, NBT * OUT], f16, tag="db0", name="db0")
            nc.sync.dma_start(out=db0[:], in_=dbar0B[:])
            # cf16/cf32 are loaded after the first GEMM-critical chunks
            # (see below) so the first matmul isn't queued behind them.

            halfI = cf16[:, C_HALFI:C_HALFI + 128]
            w2t2r = cf16[:, C_W2T2R:C_W2T2R + HID]
            w2c2 = [cf16[:, C_W2C2 + OUT * c:C_W2C2 + OUT * (c + 1)]
                    for c in range(HCH)]
            i10q = cf16[:, C_I10Q:C_I10Q + OUT]
            i10 = cf16[0:OUT, C_I10:C_I10 + OUT]
            i128 = cf16[:, C_I128:C_I128 + 128]
            idxcol = cf32[:, 0:1]
            b2col = cf32[:, 1:2]
            b1c = [cf32[:, 2 + c:3 + c] for c in range(HCH)]

            chv = [ch[:, BLOC * c:BLOC * (c + 1)] for c in range(HCH)]

            # ---------- setup: c_h = x@W1 + b1 (feature-major), s0, dbar ----
            s_cur = [sp.tile([128, BLOC], f16, tag=f"s{c}", name=f"s{c}")
                     for c in range(HCH)]
            with (
                tc.tile_pool(name="xstage", bufs=1) as xst,
                tc.tile_pool(name="w1p", bufs=1) as w1p,
                tc.tile_pool(name="dstage", bufs=2) as dst,
                tc.tile_pool(name="pS", bufs=8, space="PSUM") as pS,
            ):
                w1t = w1p.tile([128, KIN * HID], f16, tag="w1", name="w1")
                xg = xst.tile([128, KIN * BLOC], f16, tag="xg", name="xg")
                # k-outer GEMM: 8 open psum tiles per 1024-col group, so
                # matmuls start as soon as the first w1/x chunks land.
                for k in range(KIN):
                    nc.sync.dma_start(
                        out=w1t[0:KC, HID * k:HID * (k + 1)],
                        in_=w116[:, HID * k:HID * (k + 1)],
                    )
                    nc.sync.dma_start(
                        out=xg[0:KC, BLOC * k:BLOC * k + 1024],
                        in_=xT16[KC * k:KC * (k + 1), 0:1024],
                    )
                nc.sync.dma_start(out=cf16[:], in_=cst16[:])
                nc.sync.dma_start(out=cf32[:], in_=cst32[:])
                for gg in range(1, 4):
                    for k in range(KIN):
                        nc.sync.dma_start(
                            out=xg[0:KC, BLOC * k + 1024 * gg:
                                   BLOC * k + 1024 * (gg + 1)],
                            in_=xT16[KC * k:KC * (k + 1),
                                     1024 * gg:1024 * (gg + 1)],
                        )
                for c in range(HCH):
                    nc.sync.dma_start(
                        out=s_cur[c][:], in_=hT16[128 * c:128 * (c + 1), :]
                    )
                for gg in range(4):
                    qc = slice(1024 * gg, 1024 * (gg + 1))
                    pss = [pS.tile([128, 512], f32, tag="pS", name="pS")
                           for _ in range(8)]
                    for k in range(KIN):
                        for c in range(HCH):
                            for g2 in range(2):
                                nc.tensor.matmul(
                                    pss[2 * c + g2][:],
                                    w1t[0:KC, HID * k + 128 * c:
                                        HID * k + 128 * (c + 1)],
                                    xg[0:KC, BLOC * k + 1024 * gg + 512 * g2:
                                       BLOC * k + 1024 * gg + 512 * (g2 + 1)],
                                    start=(k == 0),
                                    stop=(k == KIN - 1),
                                    tile_position=(0, 0),
                                )
                    for c in range(HCH):
                        for g2 in range(2):
                            nc.scalar.activation(
                                chv[c][:, 1024 * gg + 512 * g2:
                                       1024 * gg + 512 * (g2 + 1)],
                                pss[2 * c + g2][:],
                                Act.Identity,
                                bias=b1c[c],
                                scale=1.0,
                            )
                    # s0 += c_h for this column group (SWDGE accum-DMA,
                    # keeps DVE free at the setup->loop transition)
                    for c in range(HCH):
                        nc.gpsimd.dma_start(
                            out=s_cur[c][:, qc],
                            in_=chv[c][:, qc],
                            accum_op=Alu.add,
                        )

                # dbarB (batch-major): 0.5*(c_h@W2) per batch tile,
                # then dbarB = db0 - that.  After the GEMM so the psum pool
                # rotation never stalls a column group.
                for gg in range(4):
                    ps2 = pS.tile([128, 512], f32, tag="pS", name="pS")
                    for bt in range(8 * gg, 8 * gg + 8):
                        slb = ps2[:, OUT * (bt - 8 * gg):OUT * (bt - 8 * gg + 1)]
                        for c in range(HCH):
                            nc.tensor.matmul(
                                slb,
                                chv[c][:, 128 * bt:128 * (bt + 1)],
                                w2c2[c],
                                start=(c == 0),
                                stop=(c == HCH - 1),
                                tile_position=(0, 0),
                            )
                    nc.vector.tensor_tensor(
                        dbarB[:, OUT * 8 * gg:OUT * 8 * (gg + 1)],
                        db0[:, OUT * 8 * gg:OUT * 8 * (gg + 1)],
                        ps2[:, 0:OUT * 8],
                        Alu.subtract,
                    )

            # y transpose tiles: [128, NPC*128], piece pc rows 32t+o,
            # cols p  <->  y[128*(4*pc+t)+p, o]
            yT4 = yp.tile([128, NPC * 128], f16, tag="yT4", name="yT4")
            nc.sync.dma_start(out=yT4[:], in_=y0T4[:])
            ybpad0 = yp.tile([128, NBT * 32], f16, tag="ybpad", name="ybpad")
            nc.vector.memset(ybpad0[:], 0.0)
            ybpad1 = yp.tile([128, NBT * 32], f16, tag="ybpad", name="ybpad")
            nc.vector.memset(ybpad1[:], 0.0)

            # EW path per (c, gg) tile index i = 4*c+gg:
            #   0..7  -> A1: ACT relu, DVE min, DVE add
            #   8..11 -> A2: ACT relu, Pool (min, add)
            #   12..15-> D:  DVE clip-from-psum, Pool (add)
            # ---------- relaxation loop ----------
            with (
                tc.tile_pool(name="pU", bufs=3, space="PSUM") as pU,
                tc.tile_pool(name="pY", bufs=1, space="PSUM") as pY,
                tc.tile_pool(name="pT", bufs=1, space="PSUM") as pT,
            ):
              for t in range(T):
                last = t == T - 1
                h_chunks = (0, 1, 2)
                # ---- h update (chunks 0-2 first; y-phase sits between
                # so the in-order PE queue never head-of-line blocks) ----
                s_next = (
                    None if last else
                    [sp.tile([128, BLOC], f16, tag=f"s{c}", name=f"s{c}")
                     for c in range(HCH)]
                )
                hout = (
                    [sp.tile([128, BLOC], f16, tag=f"s{c}", name=f"s{c}")
                     for c in range(HCH)] if last else None
                )
                for c in h_chunks:
                    for gg in range(4):
                        cols = slice(1024 * gg, 1024 * (gg + 1))
                        pu = pU.tile([128, 1024], f32, tag="pU", name="pU")
                        for t4 in range(8):
                            bt = 8 * gg + t4
                            band = 32 * (bt % 4)
                            ypc = bt // 4
                            nc.tensor.matmul(
                                pu[:, 128 * t4:128 * (t4 + 1)], halfI,
                                s_cur[c][:, 1024 * gg + 128 * t4:
                                          1024 * gg + 128 * (t4 + 1)],
                                start=True, stop=False, tile_position=(0, 0),
                            )
                            nc.tensor.matmul(
                                pu[:, 128 * t4:128 * (t4 + 1)],
                                w2t2r[band:band + OUT,
                                      128 * c:128 * (c + 1)],
                                yT4[band:band + OUT,
                                    128 * ypc:128 * (ypc + 1)],
                                start=False, stop=True,
                                tile_position=(band, 0),
                            )
                        i = 4 * c + gg
                        if last:
                            r = tmpp.tile([128, 1024], f16, tag="tmp",
                                          name="tmp")
                            if i % 2 == 0:
                                nc.scalar.activation(r[:], pu[:], Act.Relu)
                                nc.vector.tensor_scalar_min(
                                    hout[c][:, cols], r[:], 1.0
                                )
                            else:
                                nc.vector.tensor_scalar(
                                    hout[c][:, cols], pu[:], 0.0, 1.0,
                                    Alu.max, Alu.min,
                                )
                        elif i % 4 != 3:
                            # clip(u) into s_next; c_h added by accum-DMA below
                            r = tmpp.tile([128, 1024], f16, tag="tmp",
                                          name="tmp")
                            nc.scalar.activation(r[:], pu[:], Act.Relu)
                            nc.vector.tensor_scalar_min(
                                s_next[c][:, cols], r[:], 1.0
                            )
                        else:
                            nc.vector.tensor_scalar(
                                s_next[c][:, cols], pu[:], 0.0, 1.0,
                                Alu.max, Alu.min,
                            )
                        if not last and gg % 2 == 1:
                            # s_next += c_h over the finished 2048-col pair,
                            # on otherwise-idle DMA engines (SWDGE accum)
                            pair = slice(1024 * (gg - 1), 1024 * (gg + 1))
                            nc.gpsimd.dma_start(
                                out=s_next[c][:, pair],
                                in_=chv[c][:, pair],
                                accum_op=Alu.add,
                            )
                # ---- y update (reads s_cur, yT4) ----
                pyy = pY.tile([128, NBT * OUT], f32, tag="pY", name="pY")
                for bt in range(NBT):
                    sl = pyy[:, OUT * bt:OUT * (bt + 1)]
                    band = 32 * (bt % 4)
                    ypc = bt // 4
                    for c in range(HCH):
                        nc.tensor.matmul(
                            sl,
                            s_cur[c][:, 128 * bt:128 * (bt + 1)],
                            w2c2[c],
                            start=(c == 0),
                            stop=False,
                            tile_position=(0, 0),
                        )
                    nc.tensor.matmul(
                        sl,
                        yT4[band:band + OUT, 128 * ypc:128 * (ypc + 1)],
                        i10q[band:band + OUT, :],
                        start=False,
                        stop=False,
                        tile_position=(band, 0),
                    )
                    nc.tensor.matmul(
                        sl,
                        i128,
                        dbarB[:, OUT * bt:OUT * (bt + 1)],
                        start=False,
                        stop=True,
                        tile_position=(0, 0),
                    )
                if last:
                    ybo = tmpp.tile([128, NBT * OUT], f16, tag="ybo",
                                    name="ybo", bufs=1)
                    nc.vector.tensor_scalar(
                        ybo[:], pyy[:], 0.0, 1.0, Alu.max, Alu.min
                    )
                    nc.sync.dma_start(out=ybo16[:], in_=ybo[:])
                    yT4n = None
                else:
                    ybp = yp.tile([128, NBT * 32], f16, tag="ybpad",
                                  name="ybpad")
                    nc.vector.tensor_scalar(
                        ybp.rearrange("p (b o) -> p b o", o=32)[:, :, 0:OUT],
                        pyy.rearrange("p (b o) -> p b o", o=OUT),
                        0.0, 1.0, Alu.max, Alu.min,
                    )
                    yT4n = yp.tile([128, NPC * 128], f16, tag="yT4",
                                   name="yT4")
                    for wave in range(2):
                        pt = pT.tile([128, 512], f16, tag="pT", name="pT")
                        for j in range(4):
                            pc = 4 * wave + j
                            nc.tensor.transpose(
                                pt[:, 128 * j:128 * (j + 1)],
                                ybp[:, 128 * pc:128 * (pc + 1)],
                                i128,
                            )
                        if wave == 0:
                            nc.scalar.copy(yT4n[:, 0:512], pt[:])
                        else:
                            nc.vector.tensor_copy(yT4n[:, 512:1024], pt[:])

                h_chunks = (3,)
                for c in h_chunks:
                    for gg in range(4):
                        cols = slice(1024 * gg, 1024 * (gg + 1))
                        pu = pU.tile([128, 1024], f32, tag="pU", name="pU")
                        for t4 in range(8):
                            bt = 8 * gg + t4
                            band = 32 * (bt % 4)
                            ypc = bt // 4
                            nc.tensor.matmul(
                                pu[:, 128 * t4:128 * (t4 + 1)], halfI,
                                s_cur[c][:, 1024 * gg + 128 * t4:
                                          1024 * gg + 128 * (t4 + 1)],
                                start=True, stop=False, tile_position=(0, 0),
                            )
                            nc.tensor.matmul(
                                pu[:, 128 * t4:128 * (t4 + 1)],
                                w2t2r[band:band + OUT,
                                      128 * c:128 * (c + 1)],
                                yT4[band:band + OUT,
                                    128 * ypc:128 * (ypc + 1)],
                                start=False, stop=True,
                                tile_position=(band, 0),
                            )
                        i = 4 * c + gg
                        if last:
                            r = tmpp.tile([128, 1024], f16, tag="tmp",
                                          name="tmp")
                            if i % 2 == 0:
                                nc.scalar.activation(r[:], pu[:], Act.Relu)
                                nc.vector.tensor_scalar_min(
                                    hout[c][:, cols], r[:], 1.0
                                )
                            else:
                                nc.vector.tensor_scalar(
                                    hout[c][:, cols], pu[:], 0.0, 1.0,
                                    Alu.max, Alu.min,
                                )
                        elif i % 4 != 3:
                            # clip(u) into s_next; c_h added by accum-DMA below
                            r = tmpp.tile([128, 1024], f16, tag="tmp",
                                          name="tmp")
                            nc.scalar.activation(r[:], pu[:], Act.Relu)
                            nc.vector.tensor_scalar_min(
                                s_next[c][:, cols], r[:], 1.0
                            )
                        else:
                            nc.vector.tensor_scalar(
                                s_next[c][:, cols], pu[:], 0.0, 1.0,
                                Alu.max, Alu.min,
                            )
                        if not last and gg % 2 == 1:
                            # s_next += c_h over the finished 2048-col pair,
                            # on otherwise-idle DMA engines (SWDGE accum)
                            pair = slice(1024 * (gg - 1), 1024 * (gg + 1))
                            nc.gpsimd.dma_start(
                                out=s_next[c][:, pair],
                                in_=chv[c][:, pair],
                                accum_op=Alu.add,
                            )
                if last:
                    for c in range(HCH):
                        for hf in range(2):
                            nc.sync.dma_start(
                                out=hTo16[128 * c:128 * (c + 1),
                                          2048 * hf:2048 * (hf + 1)],
                                in_=hout[c][:, 2048 * hf:2048 * (hf + 1)],
                            )
                else:
                    s_cur = s_next
                    yT4 = yT4n

    if not nc.is_finalized():
        nc.finalize()
    return nc


def _consts(W1, W2, b1, b2):
    cst16 = np.zeros((128, CF16_W), dtype=np.float16)
    cst16[:, C_HALFI:C_HALFI + 128] = 0.5 * np.eye(128, dtype=np.float16)
    w2t2 = (0.5 * W2.T).astype(np.float16)          # [10, 512]
    w2c2 = (0.5 * W2).astype(np.float16)            # [512, 10]
    for t in range(4):
        cst16[32 * t:32 * t + OUT, C_W2T2R:C_W2T2R + HID] = w2t2
        for o in range(OUT):
            cst16[32 * t + o, C_I10Q + o] = 0.25
    for c in range(HCH):
        cst16[:, C_W2C2 + OUT * c:C_W2C2 + OUT * (c + 1)] = \
            w2c2[128 * c:128 * (c + 1), :]
    for o in range(OUT):
        cst16[o, C_I10 + o] = 1.0
    cst16[:, C_I128:C_I128 + 128] = np.eye(128, dtype=np.float16)
    cst32 = np.zeros((128, 8), dtype=np.float32)
    cst32[:, 0] = -1.0
    cst32[0:OUT, 0] = np.arange(OUT, dtype=np.float32)
    cst32[0:OUT, 1] = 0.5 * b2
    for c in range(HCH):
        cst32[:, 2 + c] = b1[128 * c:128 * (c + 1)]
    return cst16, cst32


def kernel(**inputs):
    from concourse import bass_utils

    x = np.asarray(inputs["x"], dtype=np.float32)
    h0 = np.asarray(inputs["h_init"], dtype=np.float32)
    y0 = np.asarray(inputs["y_init"], dtype=np.float32)
    W1 = np.asarray(inputs["W1"], dtype=np.float32)
    W2 = np.asarray(inputs["W2"], dtype=np.float32)
    b1 = np.asarray(inputs["b1"], dtype=np.float32).reshape(HID)
    b2 = np.asarray(inputs["b2"], dtype=np.float32).reshape(OUT)
    target = np.asarray(inputs["target"]).astype(np.int64)
    T = int(inputs["T"])

    xT16 = np.ascontiguousarray(x.T.astype(np.float16))       # [IN, B]
    hT16 = np.ascontiguousarray(h0.T.astype(np.float16))      # [HID, B]
    y016 = y0.astype(np.float16)                              # [B, OUT]
    # w116 layout: [112, 7*512], chunk k at cols 512k
    w116 = np.zeros((KC, KIN * HID), dtype=np.float16)
    for k in range(KIN):
        w116[:, HID * k:HID * (k + 1)] = W1[KC * k:KC * (k + 1), :]

    cst16, cst32 = _consts(W1, W2, b1, b2)

    key = T
    if key not in _BUILT:
        _BUILT[key] = _build(T)
    nc = _BUILT[key]

    in_maps = []
    for core in range(N_CORES):
        sl = slice(core * BLOC, (core + 1) * BLOC)
        yc = y016[sl]                                         # [BLOC, 10]
        # y0T4[32t+o, 128*pc+p] = y[128*(4*pc+t)+p, o]
        y0T4 = np.zeros((128, NPC * 128), dtype=np.float16)
        for pc in range(NPC):
            for tt in range(4):
                bt = 4 * pc + tt
                y0T4[32 * tt:32 * tt + OUT, 128 * pc:128 * (pc + 1)] = \
                    yc[128 * bt:128 * (bt + 1), :].T
        dbar0B = np.zeros((128, NBT * OUT), dtype=np.float16)
        tgt0 = target[sl]
        for bt in range(NBT):
            oh = np.zeros((128, OUT), np.float32)
            oh[np.arange(128), tgt0[128 * bt:128 * (bt + 1)]] = 1.0
            dbar0B[:, OUT * bt:OUT * (bt + 1)] = \
                (0.25 * oh + 0.5 * b2).astype(np.float16)
        in_maps.append({
            "xT16": np.ascontiguousarray(xT16[:, sl]),
            "hT16": np.ascontiguousarray(hT16[:, sl]),
            "y0T4": y0T4,
            "dbar0B": dbar0B,
            "w116": w116,
            "cst16": cst16,
            "cst32": cst32,
        })

    res = bass_utils.run_bass_kernel_spmd(nc, in_maps, list(range(N_CORES)))
    globals()["_LAST_RESULTS"] = res

    out = np.empty((B, HID + OUT), dtype=np.float32)
    for core in range(N_CORES):
        sl = slice(core * BLOC, (core + 1) * BLOC)
        out[sl, :HID] = np.asarray(res.results[core]["hTo16"]).T
        yb = np.asarray(res.results[core]["ybo16"])           # [128, 32*10]
        out[sl, HID:] = (
            yb.reshape(128, NBT, OUT).transpose(1, 0, 2).reshape(BLOC, OUT)
        )
    return out
